# revision 2
# baseline (speedup 1.0000x reference)
# DIFFormerConv (linear attention) Trainium2 kernel — 8-core data-parallel.
#
# Math (per head h, D=64, H=4, N nodes):
#   q = x@Wq^T + bq ; k = x@Wk^T + bk ; v = x@Wv^T + bv
#   qn = q/|q|, kn = k/|k|  (row-wise per head)
#   kvs = sum_l kn_l (x) v_l   [H,D,D];  vs = sum_l v_l;  ks = sum_l kn_l
#   out = mean_h (qn@kvs + vs) / (qn.ks + N)
#
# On-chip identities:
#   * q-normalization cancels in the ratio: with r=|q|,
#       (qn@kvs+vs)/(qn.ks+N) == (q@kvs + r*vs)/(q.ks + r*N)
#     so q is never normalized; r rides as column 64 of the qa store and
#     the per-head factor 0.25/denom is multiplied into qa BEFORE the
#     final matmul, which then accumulates all 4 heads into one PSUM tile.
#   * v-bias deferred: kvs = kvs0 + ks (x) bv, vs = vs0 + N*bv (post-AllReduce).
#
# x arrives pre-transposed+bf16 from the host ([IN, NL]) so the main loop
# has no on-chip transposes or casts. The only DMA-transposes (qa -> qaT
# for the final matmul) are confined to phase D behind an all-engine
# barrier, with ACT/GPSIMD fence ops so every DMA-class instruction needs
# at most one sync wait (XPOSE descriptors have a single wait slot).

import numpy as np

N_FULL = 100000
IN = 256
D = 64
H = 4
HD = 256
NCORES = 8
NLOC = N_FULL // NCORES          # 12500
NT = 98                          # l-tiles of 128 (padded)
NL = NT * 128                    # 12544
NG = NT // 2                     # 49 groups of 256 nodes
PAD0 = NLOC - (NT - 1) * 128     # used rows in last tile = 84

_nc_cache = None
_last_result = None


def _build_nc():
    from contextlib import ExitStack

    import concourse.bass as bass
    import concourse.mybir as mybir
    import concourse.tile as tile
    from concourse import bacc
    from concourse.bass import ds
    from concourse.masks import make_identity
    from bass_rust import add_dep_helper

    f32 = mybir.dt.float32
    bf16 = mybir.dt.bfloat16

    nc = bacc.Bacc()

    xT = nc.dram_tensor("xT", [IN, NL], bf16, kind="ExternalInput")
    wqT = nc.dram_tensor("wqT", [IN, HD], f32, kind="ExternalInput")
    wkT = nc.dram_tensor("wkT", [IN, HD], f32, kind="ExternalInput")
    wvT = nc.dram_tensor("wvT", [IN, HD], f32, kind="ExternalInput")
    bq = nc.dram_tensor("bq", [1, HD], f32, kind="ExternalInput")
    bk = nc.dram_tensor("bk", [1, HD], f32, kind="ExternalInput")
    bv = nc.dram_tensor("bv", [1, HD], f32, kind="ExternalInput")
    padmask = nc.dram_tensor("padmask", [128, 1], f32, kind="ExternalInput")
    out = nc.dram_tensor("out", [NL, D], f32, kind="ExternalOutput")

    with tile.TileContext(nc) as tc, ExitStack() as ctx:
        consts = ctx.enter_context(tc.tile_pool(name="consts", bufs=1))
        xtpool = ctx.enter_context(tc.tile_pool(name="xtpool", bufs=3))
        knvpool = ctx.enter_context(tc.tile_pool(name="knvpool", bufs=3))
        spool = ctx.enter_context(tc.tile_pool(name="spool", bufs=3))
        qapool = ctx.enter_context(tc.tile_pool(name="qapool", bufs=1))
        post = ctx.enter_context(tc.tile_pool(name="post", bufs=1))
        dpool = ctx.enter_context(tc.tile_pool(name="dpool", bufs=3))
        qappool = ctx.enter_context(tc.tile_pool(name="qappool", bufs=3))
        qatpool = ctx.enter_context(tc.tile_pool(name="qatpool", bufs=2))
        opool = ctx.enter_context(tc.tile_pool(name="opool", bufs=3))

        ps_q = ctx.enter_context(tc.tile_pool(name="ps_q", bufs=2, space="PSUM"))
        ps_k = ctx.enter_context(tc.tile_pool(name="ps_k", bufs=2, space="PSUM"))
        ps_v = ctx.enter_context(tc.tile_pool(name="ps_v", bufs=1, space="PSUM"))
        ps_acc = ctx.enter_context(tc.tile_pool(name="ps_acc", bufs=1, space="PSUM"))
        dram = ctx.enter_context(tc.tile_pool(name="dram", bufs=1, space="DRAM"))

        # ---- constants -------------------------------------------------
        w_sb = []
        for wTd in (wqT, wkT, wvT):
            t = consts.tile(
                [128, 2, HD], bf16, tag=f"w_{wTd.name}", name=f"w_{wTd.name}"
            )
            nc.gpsimd.dma_start(
                out=t, in_=wTd[:, :].rearrange("(cb p) f -> p cb f", p=128)
            )
            w_sb.append(t)
        wq_sb, wk_sb, wv_sb = w_sb

        bq_sb = consts.tile([1, HD], bf16, tag="bq")
        nc.gpsimd.dma_start(out=bq_sb, in_=bq[:, :])
        bk_sb = consts.tile([1, HD], bf16, tag="bk")
        nc.gpsimd.dma_start(out=bk_sb, in_=bk[:, :])
        bv_bc = consts.tile([64, HD], f32, tag="bv_bc")
        bv_ap = bv[:, :]
        nc.gpsimd.dma_start(
            out=bv_bc,
            in_=bass.AP(
                tensor=bv_ap.tensor, offset=bv_ap.offset, ap=[[0, 64]] + bv_ap.ap[1:]
            ),
        )
        bv_row = consts.tile([1, HD], f32, tag="bv_row")
        nc.gpsimd.dma_start(out=bv_row, in_=bv[:, :])
        padmask_sb = consts.tile([128, 1], f32, tag="padmask_sb")
        nc.sync.dma_start(out=padmask_sb, in_=padmask[:, :])

        ones_row = consts.tile([1, 128], bf16, tag="ones_row")
        nc.vector.memset(ones_row, 1.0)
        ones_col = consts.tile([128, 1], bf16, tag="ones_col")
        nc.vector.memset(ones_col, 1.0)
        ident64 = consts.tile([64, 64], f32, tag="ident64")
        make_identity(nc, ident64)

        # persistent qa store: [p, g, t, h, 128]; cols 0:64 = q (biased),
        # col 64 = r = |q|, cols 65:128 junk (transpose padding)
        qa_big = qapool.tile([128, NG, 2, H, 128], bf16, tag="qa_big")

        kv01_ps = ps_acc.tile([128, HD], f32, tag="kv01")
        kv23_ps = ps_acc.tile([128, HD], f32, tag="kv23")
        sums_ps = ps_acc.tile([1, 512], f32, tag="sums")

        # ---- main per-node loop ---------------------------------------
        for g in range(NG):
            xt_g = xtpool.tile([128, 2, 256], bf16, tag="xt_g", name=f"xt_{g}")
            nc.scalar.dma_start(
                out=xt_g,
                in_=xT[:, ds(g * 256, 256)].rearrange("(cb c) l -> c cb l", c=128),
            )

            q_ps = ps_q.tile([128, 2, HD], f32, tag="q_ps", name=f"q_ps_{g}")
            k_ps = ps_k.tile([128, 2, HD], f32, tag="k_ps", name=f"k_ps_{g}")
            v_ps = ps_v.tile([128, 2, HD], f32, tag="v_ps", name=f"v_ps_{g}")
            for t in range(2):
                for cb in range(2):
                    st = cb == 0
                    lhs = xt_g[:, cb, ds(t * 128, 128)]
                    nc.tensor.matmul(
                        q_ps[:, t, :], lhs, wq_sb[:, cb, :], start=st, stop=False
                    )
                    nc.tensor.matmul(
                        k_ps[:, t, :], lhs, wk_sb[:, cb, :], start=st, stop=False
                    )
                    nc.tensor.matmul(
                        v_ps[:, t, :], lhs, wv_sb[:, cb, :], start=st, stop=(cb == 1)
                    )
                nc.tensor.matmul(q_ps[:, t, :], ones_row, bq_sb, start=False, stop=True)
                nc.tensor.matmul(k_ps[:, t, :], ones_row, bk_sb, start=False, stop=True)

            qa4 = qa_big[:, g, :, :, :]  # [128, 2, H, 128]

            # --- q epilogue: evacuate (ACT), r = |q| ---
            nc.scalar.copy(
                out=qa4[:, :, :, 0:64],
                in_=q_ps.rearrange("p t (h d) -> p t h d", h=H),
            )
            sq = spool.tile([128, 2, H, 64], f32, tag="sq", name=f"sq_{g}")
            nc.vector.tensor_mul(sq, qa4[:, :, :, 0:64], qa4[:, :, :, 0:64])
            r2 = spool.tile([128, 2, H, 1], f32, tag="r2", name=f"r2_{g}")
            nc.vector.tensor_reduce(
                r2, sq, axis=mybir.AxisListType.X, op=mybir.AluOpType.add
            )
            rq = spool.tile([128, 2, H, 1], f32, tag="rq", name=f"rq_{g}")
            nc.scalar.sqrt(rq, r2)
            nc.vector.tensor_copy(qa4[:, :, :, 64:65], rq)

            # --- k epilogue: evacuate, w=1/|k|, kn = k*w ---
            kbf = spool.tile([128, 2, HD], bf16, tag="kbf", name=f"kbf_{g}")
            nc.vector.tensor_copy(kbf, k_ps)
            kbf4 = kbf.rearrange("p t (h d) -> p t h d", h=H)
            sqk = spool.tile([128, 2, H, 64], f32, tag="sqk", name=f"sqk_{g}")
            nc.vector.tensor_mul(sqk, kbf4, kbf4)
            rk2 = spool.tile([128, 2, H, 1], f32, tag="rk2", name=f"rk2_{g}")
            nc.vector.tensor_reduce(
                rk2, sqk, axis=mybir.AxisListType.X, op=mybir.AluOpType.add
            )
            rk = spool.tile([128, 2, H, 1], f32, tag="rk", name=f"rk_{g}")
            nc.scalar.sqrt(rk, rk2)
            wk_s = spool.tile([128, 2, H, 1], f32, tag="wk_s", name=f"wk_s_{g}")
            nc.vector.reciprocal(wk_s, rk)

            knv = knvpool.tile([128, 2, 512], bf16, tag="knv", name=f"knv_{g}")
            for t in range(2):
                for h in range(H):
                    nc.vector.tensor_scalar_mul(
                        knv[:, t, ds(h * 64, 64)],
                        kbf4[:, t, h, :],
                        wk_s[:, t, h, :],
                    )
            if g == NG - 1:
                # zero kn on pad rows so ks_sum/kvs stay exact
                nc.vector.tensor_scalar_mul(
                    knv[:, 1, 0:HD], knv[:, 1, 0:HD], padmask_sb
                )
            nc.scalar.copy(out=knv[:, :, ds(HD, HD)], in_=v_ps)

            # --- phase B ---
            first = g == 0
            last = g == NG - 1
            for t in range(2):
                st = first and t == 0
                sp = last and t == 1
                nc.tensor.matmul(
                    kv01_ps, knv[:, t, 0:128], knv[:, t, ds(HD, HD)], start=st, stop=sp
                )
                nc.tensor.matmul(
                    kv23_ps, knv[:, t, ds(128, 128)], knv[:, t, ds(HD, HD)],
                    start=st, stop=sp,
                )
                nc.tensor.matmul(sums_ps, ones_col, knv[:, t, :], start=st, stop=sp)

        # ---- AllReduce ------------------------------------------------
        ar_sb = post.tile([128, 512], f32, tag="ar_sb")
        nc.vector.tensor_copy(ar_sb[:, 0:HD], kv01_ps)
        nc.vector.tensor_copy(ar_sb[:, ds(HD, HD)], kv23_ps)
        sums_sb = post.tile([1, 512], f32, tag="sums_sb")
        nc.vector.tensor_copy(sums_sb, sums_ps)

        ar_in = dram.tile([129, 512], f32, tag="ar_in")
        ar_out = dram.tile([129, 512], f32, tag="ar_out")
        d1 = nc.sync.dma_start(out=ar_in[0:128, :], in_=ar_sb)
        d2 = nc.sync.dma_start(out=ar_in[128:129, :], in_=sums_sb)
        pnop = nc.gpsimd.nop()
        add_dep_helper(pnop.ins, d1.ins, sync=True, reason="absorb ar_in dma 1")
        add_dep_helper(pnop.ins, d2.ins, sync=True, reason="absorb ar_in dma 2")
        nc.gpsimd.collective_compute(
            "AllReduce",
            mybir.AluOpType.add,
            ins=[ar_in[:, :].opt()],
            outs=[ar_out[:, :].opt()],
            replica_groups=[list(range(NCORES))],
        )

        # ---- post-reduce fixups (tiny) --------------------------------
        sumr = post.tile([1, 512], f32, tag="sumr")
        nc.sync.dma_start(out=sumr, in_=ar_out[128:129, :])
        ks_cols = post.tile([64, H, 1], f32, tag="ks_cols")
        for h in range(H):
            nc.sync.dma_start(out=ks_cols[:, h, :], in_=ar_out[128:129, ds(h * 64, 64)])

        # kvs blocks -> [65, h, 64]: rows 0:64 kvs_h, row 64 vs_h
        kvsb = post.tile([65, H, 64], f32, tag="kvsb")
        blk = [(0, 0), (64, 64), (0, HD + 128), (64, HD + 192)]
        for h in range(H):
            r0, c0 = blk[h]
            nc.sync.dma_start(out=kvsb[0:64, h, :], in_=ar_out[ds(r0, 64), ds(c0, 64)])
        # kvs += ks (x) bv
        tmpo = post.tile([64, H, 64], f32, tag="tmpo")
        for h in range(H):
            nc.vector.tensor_scalar_mul(
                tmpo[:, h, :], bv_bc[:, ds(h * 64, 64)], ks_cols[:, h, :]
            )
        nc.vector.tensor_add(kvsb[0:64, :, :], kvsb[0:64, :, :], tmpo)
        # vs row = vs0 + N*bv (at partition 0, then DMA to partition 64)
        vs_tmp = post.tile([1, HD], f32, tag="vs_tmp")
        nc.vector.tensor_scalar(
            vs_tmp, bv_row, float(N_FULL), None, op0=mybir.AluOpType.mult
        )
        nc.vector.tensor_add(vs_tmp, vs_tmp, sumr[:, ds(HD, HD)])
        vs_stage = dram.tile([1, HD], f32, tag="vs_stage")
        nc.sync.dma_start(out=vs_stage, in_=vs_tmp)
        for h in range(H):
            nc.sync.dma_start(out=kvsb[64:65, h, :], in_=vs_stage[:, ds(h * 64, 64)])

        lhsT_bf = post.tile([65, H, 64], bf16, tag="lhsT_bf")
        nc.vector.tensor_copy(lhsT_bf, kvsb)

        # denom consts row: per head [4*ks_h (64) | 4*N], bcast to 128 parts
        dc = post.tile([1, H, 65], f32, tag="dc")
        for h in range(H):
            nc.vector.tensor_scalar(
                dc[:, h, 0:64], sumr[:, ds(h * 64, 64)], 4.0, None,
                op0=mybir.AluOpType.mult,
            )
        nc.vector.memset(dc[:, :, 64:65], 4.0 * N_FULL)
        dcb16 = post.tile([1, H, 65], bf16, tag="dcb16")
        nc.vector.tensor_copy(dcb16, dc)
        dcb = post.tile([128, H, 65], bf16, tag="dcb")
        nc.gpsimd.partition_broadcast(dcb, dcb16)

        # everything before this point is fully drained; phase D's
        # DMA-transposes start from a clean slate
        tc.strict_bb_all_engine_barrier()

        # ---- phase D --------------------------------------------------
        # All DMA-transposes live here, behind the barrier, with no DMA
        # copies interleaved (output accumulates in SBUF; single store at
        # the end). ACT fence-copies absorb WAW deps so each XPOSE carries
        # exactly one sync wait.
        n_full_chunks = NG // 2
        chunk_groups = [(2 * c, 2) for c in range(n_full_chunks)]
        if NG % 2:
            chunk_groups.append((NG - 1, 1))

        fscr = consts.tile([1, H, 1], bf16, tag="fscr")
        outall = qapool.tile([128, NT, 64], f32, tag="outall")
        prev_qaT = {}  # slot age 2 ring: chunk index -> list of qaT tiles

        for ci, (g0, glen) in enumerate(chunk_groups):
            width = glen * 256
            qap_b = qappool.tile(
                [128, H, 4, 128], bf16, tag="qap", name=f"qap_{g0}"
            )
            for gi in range(glen):
                g = g0 + gi
                qa4 = qa_big[:, g, :, :, :]
                for t in range(2):
                    lb = gi * 2 + t
                    prod = dpool.tile(
                        [128, H, 65], f32, tag="prod", name=f"prod_{g}_{t}"
                    )
                    nc.vector.tensor_mul(prod, qa4[:, t, :, 0:65], dcb)
                    den = dpool.tile([128, H, 1], f32, tag="den", name=f"den_{g}_{t}")
                    nc.vector.tensor_reduce(
                        den, prod, axis=mybir.AxisListType.X, op=mybir.AluOpType.add
                    )
                    rec = dpool.tile([128, H, 1], f32, tag="rec", name=f"rec_{g}_{t}")
                    nc.vector.reciprocal(rec, den)
                    for h in range(H):
                        nc.vector.tensor_scalar_mul(
                            qap_b[:, h, lb, 0:65], qa4[:, t, h, 0:65], rec[:, h, :]
                        )
                if glen == 1:
                    # tail chunk: fill the unused half with zeros so the
                    # XPOSE reads defined data
                    pass

            qaT = [
                qatpool.tile(
                    [128, 4, 128], bf16, tag=f"qaT{h}", name=f"qaT{h}_{g0}"
                )
                for h in range(H)
            ]
            for h in range(H):
                if ci >= 2:
                    fc = nc.scalar.copy(fscr[:, h, :], prev_qaT[ci - 2][h][0:1, 0:1, 0:1])
                tp = nc.scalar.dma_start(
                    out=qaT[h][:, 0 : glen * 2, :],
                    in_=qap_b[:, h, 0 : glen * 2, :],
                    transpose=True,
                )
                if ci >= 2:
                    add_dep_helper(
                        tp.ins, fc.ins, sync=False, reason="fence before xpose"
                    )
            prev_qaT[ci] = qaT

            outT_ps = ps_q.tile([64, 512], f32, tag="q_ps", name=f"outT_ps_{g0}")
            for h in range(H):
                nc.tensor.matmul(
                    outT_ps[:, 0:width],
                    lhsT_bf[:, h, :],
                    qaT[h][0:65, 0 : glen * 2, :],
                    start=(h == 0),
                    stop=(h == H - 1),
                )
            outsb = opool.tile([64, 512], f32, tag="outsb", name=f"outsb_{g0}")
            nc.scalar.copy(outsb[:, 0:width], outT_ps[:, 0:width])
            outn_ps = ps_k.tile([128, 4, 64], f32, tag="k_ps", name=f"outn_ps_{g0}")
            for lb in range(glen * 2):
                nc.tensor.transpose(
                    outn_ps[:, lb, :], outsb[:, ds(lb * 128, 128)], ident64
                )
            nc.vector.tensor_copy(
                outall[:, ds(g0 * 2, glen * 2), :], outn_ps[:, 0 : glen * 2, :]
            )

        # single store of the whole output after a full drain
        tc.strict_bb_all_engine_barrier()
        nc.sync.dma_start(
            out=out[:, :].rearrange("(lb p) d -> p lb d", p=128),
            in_=outall,
        )

    nc.finalize()
    return nc


def _get_nc():
    global _nc_cache
    if _nc_cache is None:
        _nc_cache = _build_nc()
    return _nc_cache


def kernel(x, Wq_w, Wq_b, Wk_w, Wk_b, Wv_w, Wv_b, n_nodes=None):
    import ml_dtypes
    from concourse.bass_utils import run_bass_kernel_spmd

    x = np.asarray(x, np.float32)
    xp = np.zeros((NCORES, NL, IN), np.float32)
    xp[:, :NLOC, :] = x.reshape(NCORES, NLOC, IN)

    wqT = np.ascontiguousarray(np.asarray(Wq_w, np.float32).T)
    wkT = np.ascontiguousarray(np.asarray(Wk_w, np.float32).T)
    wvT = np.ascontiguousarray(np.asarray(Wv_w, np.float32).T)
    bq = np.asarray(Wq_b, np.float32).reshape(1, HD)
    bk = np.asarray(Wk_b, np.float32).reshape(1, HD)
    bv = np.asarray(Wv_b, np.float32).reshape(1, HD)
    pm = np.ones((128, 1), np.float32)
    pm[PAD0:, 0] = 0.0

    nc = _get_nc()
    in_maps = []
    for c in range(NCORES):
        xTb = np.ascontiguousarray(xp[c].T).astype(ml_dtypes.bfloat16)
        in_maps.append(
            {
                "xT": xTb,
                "wqT": wqT,
                "wkT": wkT,
                "wvT": wvT,
                "bq": bq,
                "bk": bk,
                "bv": bv,
                "padmask": pm,
            }
        )
    import os

    trace = bool(os.environ.get("KERNEL_TRACE"))
    stitch = bool(os.environ.get("KERNEL_TRACE_STITCH"))
    tcores = os.environ.get("KERNEL_TRACE_CORES", "0")
    trace_cores = [int(c) for c in tcores.split(",")] if trace else None
    res = run_bass_kernel_spmd(
        nc,
        in_maps,
        core_ids=list(range(NCORES)),
        trace=trace,
        trace_cores=trace_cores,
        stitch_traces=stitch,
    )
    global _last_result
    _last_result = res
    outs = [res.results[c]["out"][:NLOC, :] for c in range(NCORES)]
    return np.concatenate(outs, axis=0).astype(np.float32)


if __name__ == "__main__":
    rng = np.random.default_rng(0)
    s = 1.0 / np.sqrt(IN)
    inputs = {
        "x": rng.standard_normal((N_FULL, IN)).astype(np.float32),
        "Wq_w": rng.uniform(-s, s, (HD, IN)).astype(np.float32),
        "Wq_b": rng.uniform(-s, s, HD).astype(np.float32),
        "Wk_w": rng.uniform(-s, s, (HD, IN)).astype(np.float32),
        "Wk_b": rng.uniform(-s, s, HD).astype(np.float32),
        "Wv_w": rng.uniform(-s, s, (HD, IN)).astype(np.float32),
        "Wv_b": rng.uniform(-s, s, HD).astype(np.float32),
        "n_nodes": np.array([N_FULL], np.int32),
    }
    o = kernel(**inputs)
    print(o.shape, o.dtype, np.abs(o).max())

